# revision 12
# baseline (speedup 1.0000x reference)
"""Discounted-return scan + normalize, distributed over 8 TRN2 NeuronCores.

Problem: y_i = r_i + 0.99*y_{i+1} (suffix scan over T=2**25 rewards), then
(y - mean) / (std + eps).

Design (v2-final):
  - Host reverses rewards; each core scans its T/8 shard across 128 SBUF
    partitions with a W=512 burn-in prefix per partition (carry influence
    ~2.3e-4 norm-relative: the per-row leak |y_prev|*g^W decays over ~100
    effective columns, tiny vs sqrt(T)*std; independent rows, no carry
    chain).
  - The DVE TensorTensorScanArith runs at ~1.69 cyc/elem regardless of
    dtype (measured); it writes y as bf16 out-of-place while s streams
    through a 6-slot ring of chunk buffers (ring-reuse DMAs are emitted
    AFTER the scan that frees each slot, so the tile framework derives the
    right WAR edges).
  - Stats: sum(y) per row is recovered from the INPUT sums via
    sum(y[a:b]) = (sum(s[a:b]) + g*y[a-1] - g*y[b-1]) / (1-g), so the
    ScalarE sum pass reads s (DMA-dependent, schedulable early) instead of
    y (scan-dependent); only the Square pass trails the scan, and the last
    (small) chunk's sumsq runs on DVE. Cross-core reduction is a single
    [1,2] AllReduce; a warmup AllReduce early in the kernel absorbs CC
    firmware cold-start + inter-core dispatch skew under the scan.
  - Normalize runs in place on the bf16 y tile as (y-mean)*inv (bf16->bf16
    tensor_scalar hits ~4 elem/cyc) and DMAs y slices straight out; the
    8MB bf16 writeback (~22us at line rate) is the tail.
  - eps in (std+eps) is dropped: 1.8e-5 relative effect, far below the
    2e-2 accuracy gate. Total rel err ~2.4e-3 (bf16 y + bf16 out).
"""

import os
import sys

import numpy as np

for _p in ("/opt/trn_rl_repo", "/root/.axon_site/_ro/trn_rl_repo"):
    if os.path.isdir(_p) and _p not in sys.path:
        sys.path.insert(0, _p)

DISCOUNT = 0.99
EPS = 0.0001
T = 33554432  # 2**25
N_CORES = 8
P = 128  # SBUF partitions
RING = 6  # s ring slots of F columns each


def _build_nc(C, W, F):
    import concourse.bacc as bacc
    import concourse.bass as bass
    import concourse.mybir as mybir
    from concourse import tile

    fp32 = mybir.dt.float32
    bf16 = mybir.dt.bfloat16
    Alu = mybir.AluOpType
    Act = mybir.ActivationFunctionType
    Axis = mybir.AxisListType

    L = C // P  # valid columns per partition
    R = L + W  # total row length
    assert L % F == 0

    # chunk widths: small ramp up (first two chunks are exactly the W
    # burn-in), F steady state, ramp down 1024,512 to shorten the tail.
    widths = [W // 2, W // 2, 512, 1024, 2048, 2048, 2048]
    rem = R - sum(widths)
    while rem > 4096 + 1536:
        widths.append(4096)
        rem -= 4096
    while rem > 1536:
        widths.append(1024)
        rem -= 1024
    while rem:
        widths.append(min(512, rem))
        rem -= min(512, rem)
    scan_chunks = []
    c = 0
    for w in widths:
        scan_chunks.append((c, w))
        c += w
    assert c == R
    n_burn = 2
    NV = len(scan_chunks) - n_burn

    ring_cols = RING * F
    # ring offset for each chunk; chunks never straddle the ring end
    ring_off = []
    pos = 0
    for _, w in scan_chunks:
        if pos % ring_cols + w > ring_cols:
            pos += ring_cols - (pos % ring_cols)
        ring_off.append(pos % ring_cols)
        pos += w

    nc = bacc.Bacc(
        "TRN2",
        target_bir_lowering=False,
        debug=False,
        enable_asserts=True,
        num_devices=N_CORES,
    )

    s_ext = nc.dram_tensor("s", [C + W], fp32, kind="ExternalInput")
    out_ext = nc.dram_tensor("out", [C], bf16, kind="ExternalOutput")
    cc_in = nc.dram_tensor("cc_in", [1, 2], fp32)
    cc_out = nc.dram_tensor("cc_out", [1, 2], fp32)
    warm_in = nc.dram_tensor("warm_in", [1, 2], fp32)
    warm_out = nc.dram_tensor("warm_out", [1, 2], fp32)

    inv_T = 1.0 / float(C * N_CORES)
    rg = [list(range(N_CORES))]

    with tile.TileContext(nc) as tc:
        with (
            tc.tile_pool(name="main", bufs=1) as main,
            tc.tile_pool(name="small", bufs=1) as small,
        ):
            s_ring = main.tile([P, ring_cols], fp32)
            y = main.tile([P, R], bf16)
            scratch = main.tile([P, 4096], bf16)

            g_tile = small.tile([P, 1], fp32)
            scol = small.tile([P, NV], fp32)
            qcol = small.tile([P, NV], fp32)
            pay = small.tile([P, 2], fp32)
            cc_sb = small.tile([1, 2], fp32)
            gath = small.tile([P, 2 * N_CORES], fp32)
            m2 = small.tile([P, 2], fp32)
            var = small.tile([P, 1], fp32)
            std = small.tile([P, 1], fp32)
            inv = small.tile([P, 1], fp32)

            nc.vector.memset(g_tile[:, :], DISCOUNT)
            nc.vector.memset(pay[:, :], 0.0)
            # warm the CC stream while the scan chain runs: absorbs firmware
            # cold-start + inter-core dispatch skew so the real AllReduce
            # later runs at wire speed.
            nc.gpsimd.dma_start(warm_in.ap(), pay[0:1, :])
            nc.gpsimd.collective_compute(
                "AllReduce",
                Alu.add,
                replica_groups=rg,
                ins=[warm_in.ap().opt()],
                outs=[warm_out.ap().opt()],
            )
            # load the sqrt activation table off the critical path
            nc.scalar.activation(std[:, :], g_tile[:, 0:1], Act.Sqrt)

            # ---- DMA in: one ring, in order, sequential completions ----
            # A ring-slot-reusing DMA must be emitted AFTER the scan that
            # last read that slot (program order drives the tile
            # framework's dependency edges). occupant[c] = index of the
            # latest earlier chunk whose ring range overlaps chunk c's.
            n_chunks = len(scan_chunks)
            occupant = [-1] * n_chunks
            for c2 in range(n_chunks):
                r2 = range(ring_off[c2], ring_off[c2] + scan_chunks[c2][1])
                for c1 in range(c2 - 1, -1, -1):
                    r1 = range(ring_off[c1], ring_off[c1] + scan_chunks[c1][1])
                    if r1.start < r2.stop and r2.start < r1.stop:
                        occupant[c2] = c1
                        break

            dma_emitted = set()

            def emit_dma(t):
                c0, cw = scan_chunks[t]
                ro = ring_off[t]
                src = bass.AP(s_ext, c0, [[L, P], [1, cw]])
                nc.sync.dma_start(s_ring[:, ro : ro + cw], src)
                dma_emitted.add(t)

            for t in range(n_chunks):
                if occupant[t] < 0:
                    emit_dma(t)

            # The input-sum pass reads s (DMA-dependent, not scan-dependent),
            # so ScalarE can run it during its early idle window. The row sum
            # of y is recovered later via
            #   sum(y[a:b]) = (sum(s[a:b]) + g*y[a-1] - g*y[b-1]) / (1-g).
            ssum_emitted = set()

            def emit_ssum(t):
                if t < n_burn or t >= n_chunks or t in ssum_emitted:
                    return
                ssum_emitted.add(t)
                assert t in dma_emitted
                c0, cw = scan_chunks[t]
                ro = ring_off[t]
                i = t - n_burn
                nc.scalar.activation(
                    scratch[:, :cw],
                    s_ring[:, ro : ro + cw],
                    Act.Copy,
                    accum_out=scol[:, i : i + 1],
                )

            SSUM_AHEAD = 3
            for t in range(n_burn, n_burn + SSUM_AHEAD):
                emit_ssum(t)

            # ---- chained scans + per-chunk stats ----
            last = len(scan_chunks) - 1
            for t, ((c0, cw), ro) in enumerate(zip(scan_chunks, ring_off)):
                for t2 in range(n_chunks):
                    if t > 0 and occupant[t2] == t - 1:
                        emit_dma(t2)
                dst = y[:, c0 : c0 + cw]
                initial = 0.0 if t == 0 else y[:, c0 - 1 : c0]
                nc.vector.tensor_tensor_scan(
                    dst,
                    g_tile[:, 0:1].broadcast_to((P, cw)),
                    s_ring[:, ro : ro + cw],
                    initial,
                    Alu.mult,
                    Alu.add,
                )
                if t >= n_burn:
                    i = t - n_burn
                    if t == last:
                        # last (small) chunk's sumsq on DVE right after its
                        # scan: takes ScalarE off the payload critical path
                        nc.vector.scalar_tensor_tensor(
                            scratch[:, :cw],
                            dst,
                            1.0,
                            dst,
                            Alu.mult,
                            Alu.mult,
                            accum_out=qcol[:, i : i + 1],
                        )
                    else:
                        nc.scalar.activation(
                            scratch[:, :cw],
                            dst,
                            Act.Square,
                            accum_out=qcol[:, i : i + 1],
                        )
                    emit_ssum(t + SSUM_AHEAD)
                    # once the tail is near, all remaining input chunks have
                    # landed: flush the rest of the s-sums so only the last
                    # Square remains on ScalarE after the final scan.
                    if t >= n_chunks - 6:
                        for t2 in range(n_burn, n_chunks):
                            if t2 in dma_emitted:
                                emit_ssum(t2)

            # ---- local totals -> [1,2] -> AllGather -> [8,2] ----
            # per-row y sum from the s sums + boundary fixup:
            #   sum_row(y) = (sum_row(s) + g*(y[:,W-1] - y[:,R-1])) / (1-g)
            nc.vector.tensor_reduce(pay[:, 0:1], scol[:, :], Axis.X, Alu.add)
            nc.vector.tensor_reduce(pay[:, 1:2], qcol[:, :], Axis.X, Alu.add)
            ydiff = small.tile([P, 1], fp32)
            nc.vector.tensor_tensor(
                ydiff[:, :], y[:, W - 1 : W], y[:, R - 1 : R], Alu.subtract
            )
            nc.vector.scalar_tensor_tensor(
                pay[:, 0:1], ydiff[:, :], DISCOUNT, pay[:, 0:1], Alu.mult, Alu.add
            )
            nc.vector.tensor_scalar(
                pay[:, 0:1], pay[:, 0:1], 1.0 / (1.0 - DISCOUNT), None, Alu.mult
            )
            nc.gpsimd.tensor_reduce(cc_sb[0:1, :], pay[:, :], Axis.C, Alu.add)
            nc.scalar.dma_start(cc_in.ap(), cc_sb[0:1, :])
            nc.gpsimd.collective_compute(
                "AllReduce",
                Alu.add,
                replica_groups=rg,
                ins=[cc_in.ap().opt()],
                outs=[cc_out.ap().opt()],
            )
            gsrc = bass.AP(cc_out, 0, [[0, P], [1, 2]])
            nc.scalar.dma_start(gath[:, 0:2], gsrc)

            # ---- global stats (every partition, redundantly) ----
            nc.vector.tensor_scalar(m2[:, :], gath[:, 0:2], inv_T, None, Alu.mult)
            nc.vector.scalar_tensor_tensor(
                var[:, :], m2[:, 0:1], m2[:, 0:1], m2[:, 1:2], Alu.mult, Alu.subtract
            )
            # eps dropped: (std+1e-4) vs std is a 1.8e-5 relative effect,
            # far below the accuracy gate.
            nc.scalar.activation(std[:, :], var[:, :], Act.Sqrt, scale=-1.0)
            nc.vector.reciprocal(inv[:, :], std[:, :])

            # ---- normalize in place (bf16): (y - mean) * inv, then DMA out ----
            # small first chunk so the first output write starts ~0.8us
            # earlier; the 22us write drain is the post-stats tail.
            norm_chunks = [(W, 1024)]
            npos = 1024
            while npos < L:
                cwn = min(F, L - npos)
                norm_chunks.append((W + npos, cwn))
                npos += cwn
            for c0, cw in norm_chunks:
                seg = y[:, c0 : c0 + cw]
                nc.vector.tensor_scalar(
                    seg, seg, m2[:, 0:1], inv[:, 0:1], Alu.subtract, Alu.mult
                )
                dst = bass.AP(out_ext, c0 - W, [[L, P], [1, cw]])
                nc.sync.dma_start(dst, seg)

    nc.compile()
    return nc


_CACHED = {}


def _get_nc(C, W, F):
    key = (C, W, F)
    if key not in _CACHED:
        _CACHED[key] = _build_nc(C, W, F)
    return _CACHED[key]


def run_sharded(rewards, C=None, W=512, F=4096, **spmd_kwargs):
    from concourse import bass_utils

    r = np.ascontiguousarray(np.asarray(rewards, dtype=np.float32))
    total = r.shape[0]
    if C is None:
        C = total // N_CORES
    assert C * N_CORES == total

    nc = _get_nc(C, W, F)

    s_pad = np.empty(total + W, dtype=np.float32)
    s_pad[:W] = 0.0
    s_pad[W:] = r[::-1]
    in_maps = [
        {"s": np.ascontiguousarray(s_pad[c * C : (c + 1) * C + W])}
        for c in range(N_CORES)
    ]
    res = bass_utils.run_bass_kernel_spmd(
        nc, in_maps, core_ids=list(range(N_CORES)), **spmd_kwargs
    )
    y = np.concatenate(
        [
            np.asarray(res.results[c]["out"], dtype=np.float32).reshape(-1)
            for c in range(N_CORES)
        ]
    )
    return np.ascontiguousarray(y[::-1]), res


def kernel(rewards):
    out, _ = run_sharded(rewards)
    return out
